# revision 60
# baseline (speedup 1.0000x reference)
"""Trainium2 Bass kernel for nn_DynamicConv.

Math (per token t):
    gen[t, :]  = e[t, :] @ W_weight.T + b_weight          # [4096] per-token conv weights
    w[t]       = gen[t].reshape(C_IN, C_OUT)
    out[t, o]  = sum_i x[t, i] * w[t, i, o] + (e[t] @ W_bias.T + b_bias)[o]

Kernel strategy (8 cores, data-parallel over the batch dim B=8):
  * Heavy compute is the gen matmul [4096 tok, 256] @ [256, 4096] per core.
    Done on TensorE in bf16 (fp32 PSUM accumulate), never materialized to HBM.
  * W columns are permuted o-major (j' = o*64 + i) on the host so that the
    per-token contraction over i reduces the *innermost* 64 elements.
  * ScalarE copies each PSUM tile to SBUF as bf16.
  * VectorE multiplies by x (broadcast over o via a stride-0 AP) and reduces
    over i with a pairwise tree (bf16 keeps the 2x DVE mode). DVE ops span
    GROUP token-tiles to amortize per-op overhead.
  * The dynamic-bias path rides TensorE: e @ W_bias.T accumulated with
    x_ext @ B_ext (B = b_weight.reshape(64,64), ones row adds b_bias, and the
    x@B term is the b_weight contribution to the i-contraction).
  * Resident weights/activations are DMA'd in column chunks ordered so the
    first tile's dependencies land first.
"""

from contextlib import ExitStack

import numpy as np
import ml_dtypes

import concourse.bass as bass
import concourse.tile as tile
from concourse import bacc, mybir
from concourse.bass_utils import run_bass_kernel_spmd

B, N = 8, 4096
C_IN, C_OUT, C_EMB = 64, 64, 256
NUM_W = C_IN * C_OUT  # 4096
P = 128  # tokens per tile (SBUF partitions)
N_TILES_FULL = N // P  # 32 token tiles per core (shard = one batch row)
GROUP = 3  # token tiles per DVE op group

BF16 = mybir.dt.bfloat16
F32 = mybir.dt.float32
BF16_NP = ml_dtypes.bfloat16
COPY = mybir.ActivationFunctionType.Copy


def build(nc, n_tiles):
    """Emit the per-core program. Token count = n_tiles * 128."""
    t_tot = n_tiles * P
    NW = NUM_W + C_OUT  # 4160: gen columns + dynamic-bias columns
    w2t_d = nc.dram_tensor("w2t", [C_EMB, NW], BF16, kind="ExternalInput")
    et_d = nc.dram_tensor("et", [C_EMB, t_tot], BF16, kind="ExternalInput")
    xb_d = nc.dram_tensor("xb", [P, n_tiles, C_IN], BF16, kind="ExternalInput")
    xte_d = nc.dram_tensor("xte", [C_IN + 1, t_tot], BF16, kind="ExternalInput")
    bex_d = nc.dram_tensor("bex", [C_IN + 1, C_OUT], BF16, kind="ExternalInput")
    out_d = nc.dram_tensor("out", [t_tot, C_OUT], F32, kind="ExternalOutput")

    with tile.TileContext(nc) as tc, ExitStack() as ctx:
        const = ctx.enter_context(tc.tile_pool(name="const", bufs=1))
        genp = ctx.enter_context(tc.tile_pool(name="gen", bufs=2))
        tmpp = ctx.enter_context(tc.tile_pool(name="tmp", bufs=2))
        outp = ctx.enter_context(tc.tile_pool(name="outp", bufs=4))
        psg = ctx.enter_context(tc.tile_pool(name="psg", bufs=3, space="PSUM"))
        psb = ctx.enter_context(tc.tile_pool(name="psb", bufs=2, space="PSUM"))

        # Resident tensors.
        w2t = [const.tile([P, NW], BF16, tag=f"w2t{k}", name=f"w2t{k}") for k in range(2)]
        et = [const.tile([P, t_tot], BF16, tag=f"et{k}", name=f"et{k}") for k in range(2)]
        xb = const.tile([P, n_tiles, C_IN], BF16, tag="xb")
        xte = const.tile([C_IN + 1, t_tot], BF16, tag="xte")
        bex = const.tile([C_IN + 1, C_OUT], BF16, tag="bex")

        # Load in column chunks, first-tile dependencies first.
        CW = 1024  # chunk width
        wchunks = [(0, CW), (CW, 2 * CW), (2 * CW, 3 * CW), (3 * CW, NW)]
        cwe = min(CW, t_tot)
        n_ec = t_tot // cwe  # chunks for token-indexed tensors
        tpc = n_tiles // n_ec  # tiles per chunk

        for k in range(2):
            nc.sync.dma_start(w2t[k][:, 0:CW], w2t_d[slice(k * P, (k + 1) * P), 0:CW])
        cs0 = slice(0, cwe)
        for k in range(2):
            nc.sync.dma_start(et[k][:, cs0], et_d[slice(k * P, (k + 1) * P), cs0])
        nc.sync.dma_start(xb[:, 0:tpc, :], xb_d[:, 0:tpc, :])
        for lo, hi in wchunks[1:]:
            for k in range(2):
                nc.sync.dma_start(w2t[k][:, lo:hi], w2t_d[slice(k * P, (k + 1) * P), lo:hi])
        nc.sync.dma_start(xte[:, cs0], xte_d[:, cs0])
        nc.sync.dma_start(bex[:], bex_d[:])
        for c in range(1, n_ec):
            cs = slice(c * cwe, (c + 1) * cwe)
            for k in range(2):
                nc.sync.dma_start(et[k][:, cs], et_d[slice(k * P, (k + 1) * P), cs])
            nc.sync.dma_start(
                xb[:, c * tpc:(c + 1) * tpc, :], xb_d[:, c * tpc:(c + 1) * tpc, :]
            )
            nc.sync.dma_start(xte[:, cs], xte_d[:, cs])

        def do_group(t0, glen):
            # gen2[t, o*64+i] = e[t] @ W2T for glen tiles -> bf16 SBUF.
            genb = genp.tile([P, GROUP * NUM_W], BF16, tag="genb")
            for u in range(glen):
                ts = bass.ts(t0 + u, P)
                for q in range(4):
                    ps = psg.tile([P, 1024], F32, tag="ps")
                    for k in range(2):
                        nc.tensor.matmul(
                            ps[:, 0:512],
                            et[k][:, ts],
                            w2t[k][:, q * 1024 + 0:q * 1024 + 512],
                            start=(k == 0),
                            stop=(k == 1),
                        )
                        nc.tensor.matmul(
                            ps[:, 512:1024],
                            et[k][:, ts],
                            w2t[k][:, q * 1024 + 512:q * 1024 + 1024],
                            start=(k == 0),
                            stop=(k == 1),
                        )
                    dst = genb[:, u * NUM_W + q * 1024:u * NUM_W + (q + 1) * 1024]
                    nc.scalar.activation(dst, ps[:], COPY)

            # Dynamic bias: e @ W_bias.T + x @ B + b_bias (ones row of xte).
            pb = psb.tile([P, GROUP * C_OUT], F32, tag="pb")
            for u in range(glen):
                ts = bass.ts(t0 + u, P)
                po = pb[:, u * C_OUT:(u + 1) * C_OUT]
                nc.tensor.matmul(po, et[0][:, ts], w2t[0][:, NUM_W:NW], start=True, stop=False)
                nc.tensor.matmul(po, et[1][:, ts], w2t[1][:, NUM_W:NW], start=False, stop=False)
                nc.tensor.matmul(po, xte[:, ts], bex[:], start=False, stop=True)

            # tmp1[t, u, o, i] = gen2[t, u, o, i] * x[t, u, i]
            genb4 = genb[:, 0:glen * NUM_W].rearrange(
                "p (u o i) -> p u o i", u=glen, i=C_IN
            )
            xv = (
                xb[:, t0:t0 + glen, :]
                .unsqueeze(2)
                .broadcast_to([P, glen, C_OUT, C_IN])
            )
            tmp1 = tmpp.tile([P, GROUP, C_OUT, C_IN], BF16, tag="t1")
            t1v = tmp1[:, 0:glen]
            nc.vector.tensor_mul(t1v, genb4, xv)

            # Pairwise tree reduction over i (innermost).
            cur = t1v
            w = C_IN // 2
            while w >= 1:
                nxt = tmpp.tile([P, GROUP, C_OUT, w], BF16, tag=f"tr{w}")
                nv = nxt[:, 0:glen]
                nc.vector.tensor_add(nv, cur[:, :, :, 0:w], cur[:, :, :, w:2 * w])
                cur = nv
                w //= 2
            cur = cur[:, :, :, 0:1]

            outs = outp.tile([P, GROUP, C_OUT], F32, tag="os")
            pbv = pb[:, 0:glen * C_OUT].rearrange("p (u o) -> p u o", u=glen)
            nc.vector.tensor_add(outs[:, 0:glen], cur[:, :, :, 0], pbv)
            dst = out_d[t0 * P:(t0 + glen) * P, :].rearrange(
                "(u p) o -> p u o", u=glen
            )
            nc.sync.dma_start(dst, outs[:, 0:glen])

        # Graduated group sizes so the DVE pipeline starts early.
        sizes = []
        rem = n_tiles
        for s in [1, 1, 2, 2, 2]:
            if rem >= s:
                sizes.append(s)
                rem -= s
        while (rem % GROUP) != 0 and sizes:
            rem += sizes.pop()
        while rem > 0:
            s = min(GROUP, rem)
            sizes.append(s)
            rem -= s
        t0 = 0
        for s in sizes:
            do_group(t0, s)
            t0 += s
    return out_d


def _prep_core_inputs(x_b, e_b, w2t, bex):
    """Per-core input marshalling: transposes/casts only (no math)."""
    t_tot = x_b.shape[0]
    n_tiles = t_tot // P
    et = np.ascontiguousarray(e_b.T).astype(BF16_NP)
    xb = np.ascontiguousarray(
        x_b.reshape(n_tiles, P, C_IN).transpose(1, 0, 2)
    ).astype(BF16_NP)
    xte = np.concatenate(
        [x_b.T, np.ones((1, t_tot), np.float32)], axis=0
    ).astype(BF16_NP)
    return {"w2t": w2t, "et": et, "xb": xb, "xte": xte, "bex": bex}


def prep_shared(W_weight, b_weight, W_bias, b_bias):
    # o-major column permutation: W2[o*64+i, c] = W_weight[i*64+o, c],
    # then W_bias.T appended as 64 extra columns (the dynamic-bias path).
    w2 = W_weight.reshape(C_IN, C_OUT, C_EMB).transpose(1, 0, 2).reshape(NUM_W, C_EMB)
    w2t = np.concatenate([w2.T, W_bias.T], axis=1)
    w2t = np.ascontiguousarray(w2t).astype(BF16_NP)
    bex = np.concatenate(
        [b_weight.reshape(C_IN, C_OUT), b_bias.reshape(1, C_OUT)], axis=0
    ).astype(BF16_NP)
    return w2t, bex


_CACHE = {}


def _get_nc(n_tiles, num_devices):
    key = (n_tiles, num_devices)
    if key not in _CACHE:
        nc = bacc.Bacc(
            "TRN2", target_bir_lowering=False, debug=False, num_devices=num_devices
        )
        build(nc, n_tiles)
        nc.compile()
        _CACHE[key] = nc
    return _CACHE[key]


def kernel(x, embed_feature, W_weight, b_weight, W_bias, b_bias, _trace=False):
    x = np.asarray(x, np.float32)
    embed_feature = np.asarray(embed_feature, np.float32)
    W_weight = np.asarray(W_weight, np.float32)
    b_weight = np.asarray(b_weight, np.float32)
    W_bias = np.asarray(W_bias, np.float32)
    b_bias = np.asarray(b_bias, np.float32)
    assert x.shape == (B, N, C_IN) and embed_feature.shape == (B, N, C_EMB)
    w2t, bex = prep_shared(W_weight, b_weight, W_bias, b_bias)
    in_maps = [
        _prep_core_inputs(x[b], embed_feature[b], w2t, bex) for b in range(B)
    ]
    nc = _get_nc(N_TILES_FULL, B)
    res = run_bass_kernel_spmd(
        nc, in_maps, list(range(B)), trace=_trace,
        trace_cores=list(range(B)) if _trace == "all" else None,
    )
    out = np.stack([res.results[b]["out"] for b in range(B)], axis=0)
    kernel.last_result = res
    return out.astype(np.float32)


# revision 62
# speedup vs baseline: 1.0030x; 1.0030x over previous
"""Trainium2 Bass kernel for nn_DynamicConv.

Math (per token t):
    gen[t, :]  = e[t, :] @ W_weight.T + b_weight          # [4096] per-token conv weights
    w[t]       = gen[t].reshape(C_IN, C_OUT)
    out[t, o]  = sum_i x[t, i] * w[t, i, o] + (e[t] @ W_bias.T + b_bias)[o]

Kernel strategy (8 cores, data-parallel over the batch dim B=8):
  * Heavy compute is the gen matmul [4096 tok, 256] @ [256, 4096] per core.
    Done on TensorE in bf16 (fp32 PSUM accumulate), never materialized to HBM.
  * W columns are permuted o-major (j' = o*64 + i) on the host so that the
    per-token contraction over i reduces the *innermost* 64 elements.
  * ScalarE copies each PSUM tile to SBUF as bf16.
  * VectorE multiplies by x (broadcast over o via a stride-0 AP) and reduces
    over i with a pairwise tree (bf16 keeps the 2x DVE mode). DVE ops span
    GROUP token-tiles to amortize per-op overhead.
  * The dynamic-bias path rides TensorE: e @ W_bias.T accumulated with
    x_ext @ B_ext (B = b_weight.reshape(64,64), ones row adds b_bias, and the
    x@B term is the b_weight contribution to the i-contraction).
  * Resident weights/activations are DMA'd in column chunks ordered so the
    first tile's dependencies land first.
"""

from contextlib import ExitStack

import numpy as np
import ml_dtypes

import concourse.bass as bass
import concourse.tile as tile
from concourse import bacc, mybir
from concourse.bass_utils import run_bass_kernel_spmd

B, N = 8, 4096
C_IN, C_OUT, C_EMB = 64, 64, 256
NUM_W = C_IN * C_OUT  # 4096
P = 128  # tokens per tile (SBUF partitions)
N_TILES_FULL = N // P  # 32 token tiles per core (shard = one batch row)
GROUP = 3  # token tiles per DVE op group

BF16 = mybir.dt.bfloat16
F32 = mybir.dt.float32
BF16_NP = ml_dtypes.bfloat16
COPY = mybir.ActivationFunctionType.Copy


def build(nc, n_tiles):
    """Emit the per-core program. Token count = n_tiles * 128."""
    t_tot = n_tiles * P
    NW = NUM_W + C_OUT  # 4160: gen columns + dynamic-bias columns
    w2t_d = nc.dram_tensor("w2t", [C_EMB, NW], BF16, kind="ExternalInput")
    et_d = nc.dram_tensor("et", [C_EMB, t_tot], BF16, kind="ExternalInput")
    xb_d = nc.dram_tensor("xb", [P, n_tiles, C_IN], BF16, kind="ExternalInput")
    xte_d = nc.dram_tensor("xte", [C_IN + 1, t_tot], BF16, kind="ExternalInput")
    bex_d = nc.dram_tensor("bex", [C_IN + 1, C_OUT], BF16, kind="ExternalInput")
    out_d = nc.dram_tensor("out", [t_tot, C_OUT], F32, kind="ExternalOutput")

    with tile.TileContext(nc) as tc, ExitStack() as ctx:
        const = ctx.enter_context(tc.tile_pool(name="const", bufs=1))
        genp = ctx.enter_context(tc.tile_pool(name="gen", bufs=2))
        tmpp = ctx.enter_context(tc.tile_pool(name="tmp", bufs=2))
        outp = ctx.enter_context(tc.tile_pool(name="outp", bufs=4))
        psg = ctx.enter_context(tc.tile_pool(name="psg", bufs=3, space="PSUM"))
        psb = ctx.enter_context(tc.tile_pool(name="psb", bufs=2, space="PSUM"))

        # Resident tensors.
        w2t = [const.tile([P, NW], BF16, tag=f"w2t{k}", name=f"w2t{k}") for k in range(2)]
        et = [const.tile([P, t_tot], BF16, tag=f"et{k}", name=f"et{k}") for k in range(2)]
        xb = const.tile([P, n_tiles, C_IN], BF16, tag="xb")
        xte = const.tile([C_IN + 1, t_tot], BF16, tag="xte")
        bex = const.tile([C_IN + 1, C_OUT], BF16, tag="bex")

        # Load in column chunks, first-tile dependencies first.
        CW = 1024  # chunk width
        wchunks = [(0, CW), (CW, 2 * CW), (2 * CW, 3 * CW), (3 * CW, NW)]
        cwe = min(CW, t_tot)
        n_ec = t_tot // cwe  # chunks for token-indexed tensors
        tpc = n_tiles // n_ec  # tiles per chunk

        # PE warmup: dummy matmuls on (never-written) SBUF fill the dead
        # window while the first DMAs land, so the HAM clock-gate is at
        # 2.4 GHz when the real matmul stream starts. Results land in a
        # rotating PSUM slot and are discarded.
        dummy = const.tile([P, 512], BF16, tag="warm")
        nc.gpsimd.memset(dummy[:], 0)
        wps = psg.tile([P, 1024], F32, tag="ps", name="warm_ps")
        for i in range(12):
            nc.tensor.matmul(
                wps[:, 0:512], dummy[:, 0:P], dummy[:], start=True, stop=True
            )

        for k in range(2):
            nc.sync.dma_start(w2t[k][:, 0:CW], w2t_d[slice(k * P, (k + 1) * P), 0:CW])
        cs0 = slice(0, cwe)
        for k in range(2):
            nc.sync.dma_start(et[k][:, cs0], et_d[slice(k * P, (k + 1) * P), cs0])
        nc.sync.dma_start(xb[:, 0:tpc, :], xb_d[:, 0:tpc, :])
        for lo, hi in wchunks[1:]:
            for k in range(2):
                nc.sync.dma_start(w2t[k][:, lo:hi], w2t_d[slice(k * P, (k + 1) * P), lo:hi])
        nc.sync.dma_start(xte[:, cs0], xte_d[:, cs0])
        nc.sync.dma_start(bex[:], bex_d[:])
        for c in range(1, n_ec):
            cs = slice(c * cwe, (c + 1) * cwe)
            for k in range(2):
                nc.sync.dma_start(et[k][:, cs], et_d[slice(k * P, (k + 1) * P), cs])
            nc.sync.dma_start(
                xb[:, c * tpc:(c + 1) * tpc, :], xb_d[:, c * tpc:(c + 1) * tpc, :]
            )
            nc.sync.dma_start(xte[:, cs], xte_d[:, cs])

        def do_group(t0, glen):
            # gen2[t, o*64+i] = e[t] @ W2T for glen tiles -> bf16 SBUF.
            genb = genp.tile([P, GROUP * NUM_W], BF16, tag="genb")
            for u in range(glen):
                ts = bass.ts(t0 + u, P)
                for q in range(4):
                    ps = psg.tile([P, 1024], F32, tag="ps")
                    for k in range(2):
                        nc.tensor.matmul(
                            ps[:, 0:512],
                            et[k][:, ts],
                            w2t[k][:, q * 1024 + 0:q * 1024 + 512],
                            start=(k == 0),
                            stop=(k == 1),
                        )
                        nc.tensor.matmul(
                            ps[:, 512:1024],
                            et[k][:, ts],
                            w2t[k][:, q * 1024 + 512:q * 1024 + 1024],
                            start=(k == 0),
                            stop=(k == 1),
                        )
                    dst = genb[:, u * NUM_W + q * 1024:u * NUM_W + (q + 1) * 1024]
                    nc.scalar.activation(dst, ps[:], COPY)

            # Dynamic bias: e @ W_bias.T + x @ B + b_bias (ones row of xte).
            pb = psb.tile([P, GROUP * C_OUT], F32, tag="pb")
            for u in range(glen):
                ts = bass.ts(t0 + u, P)
                po = pb[:, u * C_OUT:(u + 1) * C_OUT]
                nc.tensor.matmul(po, et[0][:, ts], w2t[0][:, NUM_W:NW], start=True, stop=False)
                nc.tensor.matmul(po, et[1][:, ts], w2t[1][:, NUM_W:NW], start=False, stop=False)
                nc.tensor.matmul(po, xte[:, ts], bex[:], start=False, stop=True)

            # tmp1[t, u, o, i] = gen2[t, u, o, i] * x[t, u, i]
            genb4 = genb[:, 0:glen * NUM_W].rearrange(
                "p (u o i) -> p u o i", u=glen, i=C_IN
            )
            xv = (
                xb[:, t0:t0 + glen, :]
                .unsqueeze(2)
                .broadcast_to([P, glen, C_OUT, C_IN])
            )
            tmp1 = tmpp.tile([P, GROUP, C_OUT, C_IN], BF16, tag="t1")
            t1v = tmp1[:, 0:glen]
            nc.vector.tensor_mul(t1v, genb4, xv)

            # Pairwise tree reduction over i (innermost).
            cur = t1v
            w = C_IN // 2
            while w >= 1:
                nxt = tmpp.tile([P, GROUP, C_OUT, w], BF16, tag=f"tr{w}")
                nv = nxt[:, 0:glen]
                nc.vector.tensor_add(nv, cur[:, :, :, 0:w], cur[:, :, :, w:2 * w])
                cur = nv
                w //= 2
            cur = cur[:, :, :, 0:1]

            outs = outp.tile([P, GROUP, C_OUT], F32, tag="os")
            pbv = pb[:, 0:glen * C_OUT].rearrange("p (u o) -> p u o", u=glen)
            nc.vector.tensor_add(outs[:, 0:glen], cur[:, :, :, 0], pbv)
            dst = out_d[t0 * P:(t0 + glen) * P, :].rearrange(
                "(u p) o -> p u o", u=glen
            )
            nc.sync.dma_start(dst, outs[:, 0:glen])

        # Graduated group sizes so the DVE pipeline starts early.
        sizes = []
        rem = n_tiles
        for s in [1, 1, 2, 2, 2]:
            if rem >= s:
                sizes.append(s)
                rem -= s
        while (rem % GROUP) != 0 and sizes:
            rem += sizes.pop()
        while rem > 0:
            s = min(GROUP, rem)
            sizes.append(s)
            rem -= s
        t0 = 0
        for s in sizes:
            do_group(t0, s)
            t0 += s
    return out_d


def _prep_core_inputs(x_b, e_b, w2t, bex):
    """Per-core input marshalling: transposes/casts only (no math)."""
    t_tot = x_b.shape[0]
    n_tiles = t_tot // P
    et = np.ascontiguousarray(e_b.T).astype(BF16_NP)
    xb = np.ascontiguousarray(
        x_b.reshape(n_tiles, P, C_IN).transpose(1, 0, 2)
    ).astype(BF16_NP)
    xte = np.concatenate(
        [x_b.T, np.ones((1, t_tot), np.float32)], axis=0
    ).astype(BF16_NP)
    return {"w2t": w2t, "et": et, "xb": xb, "xte": xte, "bex": bex}


def prep_shared(W_weight, b_weight, W_bias, b_bias):
    # o-major column permutation: W2[o*64+i, c] = W_weight[i*64+o, c],
    # then W_bias.T appended as 64 extra columns (the dynamic-bias path).
    w2 = W_weight.reshape(C_IN, C_OUT, C_EMB).transpose(1, 0, 2).reshape(NUM_W, C_EMB)
    w2t = np.concatenate([w2.T, W_bias.T], axis=1)
    w2t = np.ascontiguousarray(w2t).astype(BF16_NP)
    bex = np.concatenate(
        [b_weight.reshape(C_IN, C_OUT), b_bias.reshape(1, C_OUT)], axis=0
    ).astype(BF16_NP)
    return w2t, bex


_CACHE = {}


def _get_nc(n_tiles, num_devices):
    key = (n_tiles, num_devices)
    if key not in _CACHE:
        nc = bacc.Bacc(
            "TRN2", target_bir_lowering=False, debug=False, num_devices=num_devices
        )
        build(nc, n_tiles)
        nc.compile()
        _CACHE[key] = nc
    return _CACHE[key]


def kernel(x, embed_feature, W_weight, b_weight, W_bias, b_bias, _trace=False):
    x = np.asarray(x, np.float32)
    embed_feature = np.asarray(embed_feature, np.float32)
    W_weight = np.asarray(W_weight, np.float32)
    b_weight = np.asarray(b_weight, np.float32)
    W_bias = np.asarray(W_bias, np.float32)
    b_bias = np.asarray(b_bias, np.float32)
    assert x.shape == (B, N, C_IN) and embed_feature.shape == (B, N, C_EMB)
    w2t, bex = prep_shared(W_weight, b_weight, W_bias, b_bias)
    in_maps = [
        _prep_core_inputs(x[b], embed_feature[b], w2t, bex) for b in range(B)
    ]
    nc = _get_nc(N_TILES_FULL, B)
    res = run_bass_kernel_spmd(
        nc, in_maps, list(range(B)), trace=_trace,
        trace_cores=list(range(B)) if _trace == "all" else None,
    )
    out = np.stack([res.results[b]["out"] for b in range(B)], axis=0)
    kernel.last_result = res
    return out.astype(np.float32)
